# revision 8
# baseline (speedup 1.0000x reference)
"""Trainium2 Bass kernel for nn_CPUMoE (RMSNorm + top-2-of-8 MoE GLU).

Strategy (expert-parallel, host-mediated dispatch):
  Phase A (SPMD, data-parallel over tokens): each of 8 cores takes a
    1024-token shard; computes RMSNorm, router logits (graded output),
    softmax top-2 combine weights, and the transposed normed activations
    xnT [H, 1024] (PE transposes, needed because the MLP contracts over H).
  Host: reads combine weights, builds per-expert token index lists,
    gathers xnT columns per expert (pure data movement), pre-transposes
    expert weights.
  Phase B (SPMD, expert-parallel): core e runs expert e's GLU MLP over its
    (capacity-padded) tokens: g = Wg@xnT, u = Wu@xnT, h = silu(g)*u*cw,
    y = (h^T @ WdT). Streams weights; keeps token chunk resident in SBUF.
  Host: scatter-adds the two expert contributions per token (combine).

Self-contained: hardcodes all shapes from the problem spec.
"""
import os
import numpy as np
import ml_dtypes
from contextlib import ExitStack

import concourse.bass as bass
import concourse.bacc as bacc
import concourse.tile as tile
from concourse import mybir
from concourse.bass_utils import run_bass_kernel_spmd
from concourse.masks import make_identity

F32 = mybir.dt.float32
BF16 = mybir.dt.bfloat16
AX = mybir.AxisListType
ALU = mybir.AluOpType
ACTF = mybir.ActivationFunctionType

# problem dims (from spec; fixed)
B, S, H, I, E = 4, 2048, 2048, 1408, 8
T = B * S            # 8192 tokens
NCORES = 8
TA = T // NCORES     # tokens per core, phase A
EPS = 1e-6

# config
MM_DT = os.environ.get("MOE_MM_DT", "f32r")   # "f32" | "bf16" | "f32r"  (phase-B matmul dtype)
USE_SILU_LUT = os.environ.get("MOE_SILU_LUT", "1") == "1"
CAP = 2304                                    # per-expert capacity per launch
CHUNK = 1152 if MM_DT == "bf16" else 768      # token chunk resident in SBUF
HC = 512                                       # down-proj output col chunk
HB = H // 128        # 16
IB = I // 128        # 11
TB = TA // 128       # 8


def _bc(ap, n):
    """Broadcast a trailing size-1 free dim of an AP to n (step 0)."""
    a = [list(x) for x in ap.ap]
    assert a[-1][1] == 1
    a[-1] = [0, n]
    return bass.AP(tensor=ap.tensor, offset=ap.offset, ap=a)


def _bcast_row(t, p, n, offset=0):
    """DRAM 1-D tensor AP broadcast across p partitions, n elements."""
    return bass.AP(tensor=t.tensor, offset=offset, ap=[[0, p], [1, n]])


F32R = mybir.dt.float32r
_MDT = {"bf16": BF16, "f32": F32, "f32r": F32R}


def _round_f32r(x):
    """Host-side replication of the on-chip f32->f32r rounding (RNE on the low
    8 mantissa bits, keeping 15 explicit mantissa bits)."""
    b = x.view(np.uint32).astype(np.uint64)
    lsb = (b >> np.uint64(8)) & np.uint64(1)
    r = (b + np.uint64(0x7F) + lsb) & ~np.uint64(0xFF)
    return r.astype(np.uint32).view(np.float32)


def build_phase_a(silu_unused=None):
    out_dt = BF16 if MM_DT == "bf16" else F32
    nc = bacc.Bacc("TRN2", target_bir_lowering=False, debug=False)
    x = nc.dram_tensor("x", [TA, H], F32, kind="ExternalInput").ap()
    rw = nc.dram_tensor("rw", [H], F32, kind="ExternalInput").ap()
    rt = nc.dram_tensor("rt", [H, E], F32, kind="ExternalInput").ap()   # router_weight.T
    xnt = nc.dram_tensor("xnt", [H, TA], out_dt, kind="ExternalOutput").ap()
    cw = nc.dram_tensor("cw", [TA, E], F32, kind="ExternalOutput").ap()
    lg = nc.dram_tensor("lg", [TA, E], F32, kind="ExternalOutput").ap()

    x_r = x.rearrange("(tb p) h -> p tb h", p=128)
    xnt_r = xnt.rearrange("(hb p) t -> p hb t", p=128)

    with tile.TileContext(nc) as tc, ExitStack() as ctx:
        konst = ctx.enter_context(tc.tile_pool(name="konst", bufs=1))
        xp = ctx.enter_context(tc.tile_pool(name="xp", bufs=2))
        trp = ctx.enter_context(tc.tile_pool(name="trp", bufs=2))
        outp = ctx.enter_context(tc.tile_pool(name="outp", bufs=2))
        sm = ctx.enter_context(tc.tile_pool(name="sm", bufs=4))
        pers = ctx.enter_context(tc.tile_pool(name="pers", bufs=1))
        pst = ctx.enter_context(tc.tile_pool(name="pst", bufs=4, space="PSUM"))
        psl = ctx.enter_context(tc.tile_pool(name="psl", bufs=2, space="PSUM"))

        ident = konst.tile([128, 128], F32)
        make_identity(nc, ident)
        rmsb = konst.tile([128, H], F32)
        nc.sync.dma_start(out=rmsb, in_=_bcast_row(rw, 128, H))
        rwt = konst.tile([128, HB, E], F32)
        nc.sync.dma_start(out=rwt, in_=rt.rearrange("(hb p) e -> p hb e", p=128))
        epst = konst.tile([128, 1], F32)
        nc.vector.memset(epst, EPS)

        lg_all = pers.tile([128, TB, E], F32)

        for tt in range(TB):
            xt = xp.tile([128, H], F32, tag="xt")
            nc.sync.dma_start(out=xt, in_=x_r[:, tt, :])
            sq = xp.tile([128, H], F32, tag="sq")
            ssum = sm.tile([128, 1], F32)
            # (tensor_tensor_reduce crashes NRT in this env; ACT Square+accum instead)
            nc.scalar.activation(sq, xt, ACTF.Square, accum_out=ssum)
            rstd = sm.tile([128, 1], F32)
            nc.scalar.activation(rstd, ssum, ACTF.Sqrt, bias=epst, scale=1.0 / H)
            nc.vector.reciprocal(rstd, rstd)
            xs = xp.tile([128, H], F32, tag="xs")
            nc.scalar.activation(xs, xt, ACTF.Copy, scale=rstd)
            xn = xp.tile([128, H], F32, tag="xn")
            nc.vector.tensor_mul(xn, xs, rmsb)

            tr = trp.tile([128, HB, 128], F32)
            for hb in range(HB):
                ptr = pst.tile([128, 128], F32)
                nc.tensor.transpose(ptr, xn[:, hb * 128:(hb + 1) * 128], ident)
                nc.any.tensor_copy(out=tr[:, hb, :], in_=ptr)

            if out_dt == F32:
                nc.sync.dma_start(out=xnt_r[:, :, tt * 128:(tt + 1) * 128], in_=tr)
            else:
                trc = outp.tile([128, HB, 128], out_dt)
                nc.vector.tensor_copy(trc, tr)
                nc.sync.dma_start(out=xnt_r[:, :, tt * 128:(tt + 1) * 128], in_=trc)

            lg_ps = psl.tile([128, E], F32)
            for hb in range(HB):
                nc.tensor.matmul(lg_ps, tr[:, hb, :], rwt[:, hb, :],
                                 start=(hb == 0), stop=(hb == HB - 1))
            nc.any.tensor_copy(out=lg_all[:, tt, :], in_=lg_ps)

        # batched softmax-top2 combine weights over [128, TB, E]
        m = sm.tile([128, TB, 1], F32, tag="m")
        nc.vector.tensor_reduce(out=m, in_=lg_all, op=ALU.max, axis=AX.X)
        d = pers.tile([128, TB, E], F32, tag="d")
        nc.vector.tensor_tensor(out=d, in0=lg_all, in1=_bc(m, E), op=ALU.subtract)
        p = pers.tile([128, TB, E], F32, tag="p")
        nc.scalar.activation(p, d, ACTF.Exp)
        p1 = sm.tile([128, TB, 1], F32, tag="p1")
        nc.vector.tensor_reduce(out=p1, in_=p, op=ALU.max, axis=AX.X)
        m1 = pers.tile([128, TB, E], F32, tag="m1")
        nc.vector.tensor_tensor(out=m1, in0=p, in1=_bc(p1, E), op=ALU.is_ge)
        pm = pers.tile([128, TB, E], F32, tag="pm")
        nc.vector.tensor_tensor(out=pm, in0=p, in1=m1, op=ALU.mult)
        nc.vector.tensor_tensor(out=pm, in0=p, in1=pm, op=ALU.subtract)
        p2 = sm.tile([128, TB, 1], F32, tag="p2")
        nc.vector.tensor_reduce(out=p2, in_=pm, op=ALU.max, axis=AX.X)
        m2 = pers.tile([128, TB, E], F32, tag="m2")
        nc.vector.tensor_tensor(out=m2, in0=pm, in1=_bc(p2, E), op=ALU.is_ge)
        den = sm.tile([128, TB, 1], F32, tag="den")
        nc.vector.tensor_tensor(out=den, in0=p1, in1=p2, op=ALU.add)
        inv = sm.tile([128, TB, 1], F32, tag="inv")
        nc.vector.reciprocal(inv, den)
        sel = pers.tile([128, TB, E], F32, tag="sel")
        nc.vector.tensor_tensor(out=sel, in0=m1, in1=m2, op=ALU.add)
        cwt = pers.tile([128, TB, E], F32, tag="cwt")
        nc.vector.tensor_tensor(out=cwt, in0=p, in1=sel, op=ALU.mult)
        nc.vector.tensor_tensor(out=cwt, in0=cwt, in1=_bc(inv, E), op=ALU.mult)

        nc.sync.dma_start(out=lg.rearrange("(tb p) e -> p tb e", p=128), in_=lg_all)
        nc.sync.dma_start(out=cw.rearrange("(tb p) e -> p tb e", p=128), in_=cwt)

    nc.compile()
    return nc


def _col_groups(n):
    groups, n0 = [], 0
    while n0 < n:
        nn = min(512, n - n0)
        groups.append((n0, nn))
        n0 += nn
    return groups


def build_phase_b():
    mdt = _MDT[MM_DT]
    nc = bacc.Bacc("TRN2", target_bir_lowering=False, debug=False)
    xet = nc.dram_tensor("xet", [H, CAP], mdt, kind="ExternalInput").ap()
    wg = nc.dram_tensor("wg", [H, I], mdt, kind="ExternalInput").ap()
    wu = nc.dram_tensor("wu", [H, I], mdt, kind="ExternalInput").ap()
    wd = nc.dram_tensor("wd", [I, H], mdt, kind="ExternalInput").ap()
    cwv = nc.dram_tensor("cwv", [CAP], F32, kind="ExternalInput").ap()
    y = nc.dram_tensor("y", [CAP, H], F32, kind="ExternalOutput").ap()

    xet_r = xet.rearrange("(hb p) t -> p hb t", p=128)
    wg_r = wg.rearrange("(hb p) i -> p hb i", p=128)
    wu_r = wu.rearrange("(hb p) i -> p hb i", p=128)
    wd_r = wd.rearrange("(ib p) h -> p ib h", p=128)
    y_r = y.rearrange("(tb p) h -> p tb h", p=128)

    n_chunks = CAP // CHUNK
    TBC = CHUNK // 128
    NHC = H // HC
    cgs = _col_groups(CHUNK)

    with tile.TileContext(nc) as tc, ExitStack() as ctx:
        xp = ctx.enter_context(tc.tile_pool(name="xp", bufs=2 if mdt == BF16 else 1))
        hp = ctx.enter_context(tc.tile_pool(name="hp", bufs=1))
        wp = ctx.enter_context(tc.tile_pool(name="wp", bufs=2))
        dp = ctx.enter_context(tc.tile_pool(name="dp", bufs=2))
        sp = ctx.enter_context(tc.tile_pool(name="sp", bufs=2))
        cwp = ctx.enter_context(tc.tile_pool(name="cwp", bufs=2))
        yp = ctx.enter_context(tc.tile_pool(name="yp", bufs=4))
        pg = ctx.enter_context(tc.tile_pool(name="pg", bufs=1, space="PSUM"))
        py = ctx.enter_context(tc.tile_pool(name="py", bufs=2, space="PSUM"))

        for c in range(n_chunks):
            xet_sb = xp.tile([128, HB, CHUNK], mdt, tag="xet")
            nc.sync.dma_start(out=xet_sb,
                              in_=xet_r[:, :, c * CHUNK:(c + 1) * CHUNK])
            cwb = cwp.tile([128, CHUNK], F32, tag="cwb")
            nc.sync.dma_start(out=cwb,
                              in_=bass.AP(tensor=cwv.tensor, offset=c * CHUNK,
                                          ap=[[0, 128], [1, CHUNK]]))
            ht_sb = hp.tile([128, IB, CHUNK], mdt, tag="ht")
            for i in range(IB):
                wgt = wp.tile([128, HB, 128], mdt, tag="wg")
                nc.sync.dma_start(out=wgt, in_=wg_r[:, :, i * 128:(i + 1) * 128])
                wut = wp.tile([128, HB, 128], mdt, tag="wu")
                nc.sync.dma_start(out=wut, in_=wu_r[:, :, i * 128:(i + 1) * 128])
                gt = pg.tile([128, CHUNK], F32, tag="g")
                ut = pg.tile([128, CHUNK], F32, tag="u")
                for (n0, nn) in cgs:
                    for hb in range(HB):
                        nc.tensor.matmul(gt[:, n0:n0 + nn], wgt[:, hb, :],
                                         xet_sb[:, hb, n0:n0 + nn],
                                         start=(hb == 0), stop=(hb == HB - 1))
                for (n0, nn) in cgs:
                    for hb in range(HB):
                        nc.tensor.matmul(ut[:, n0:n0 + nn], wut[:, hb, :],
                                         xet_sb[:, hb, n0:n0 + nn],
                                         start=(hb == 0), stop=(hb == HB - 1))
                sg = sp.tile([128, CHUNK], F32, tag="sg")
                if USE_SILU_LUT:
                    nc.scalar.activation(sg, gt, ACTF.Silu)
                else:
                    nc.scalar.activation(sg, gt, ACTF.Sigmoid)
                    nc.vector.tensor_mul(sg, sg, gt)
                hu = sp.tile([128, CHUNK], F32, tag="hu")
                nc.vector.tensor_mul(hu, sg, ut)
                nc.vector.tensor_mul(ht_sb[:, i, :], hu, cwb)

            for hc in range(NHC):
                wdt = dp.tile([128, IB, HC], mdt, tag="wd")
                nc.sync.dma_start(out=wdt, in_=wd_r[:, :, hc * HC:(hc + 1) * HC])
                for t in range(TBC):
                    y_ps = py.tile([128, HC], F32, tag="y")
                    for i in range(IB):
                        nc.tensor.matmul(y_ps, ht_sb[:, i, t * 128:(t + 1) * 128],
                                         wdt[:, i, :],
                                         start=(i == 0), stop=(i == IB - 1))
                    ysb = yp.tile([128, HC], F32, tag="ysb")
                    nc.any.tensor_copy(out=ysb, in_=y_ps)
                    nc.sync.dma_start(
                        out=y_r[:, c * TBC + t, hc * HC:(hc + 1) * HC], in_=ysb)

    nc.compile()
    return nc


_programs = {}
_last_results = {}


def _trace_kwargs():
    if os.environ.get("MOE_TRACE", "0") != "1":
        return {}
    return {"trace": True}


def _get_program(name):
    if name not in _programs:
        _programs[name] = build_phase_a() if name == "a" else build_phase_b()
    return _programs[name]


_wcache = {}


def _prep_weights(w_gate, w_up, w_down):
    key = (id(w_gate), id(w_up), id(w_down))
    if _wcache.get("key") == key:
        return _wcache["val"]
    mnp = ml_dtypes.bfloat16 if MM_DT == "bf16" else np.float32

    def prep(w):
        return np.ascontiguousarray(w.T).astype(mnp)

    val = []
    for e in range(E):
        val.append({
            "wg": prep(np.asarray(w_gate)[e]),
            "wu": prep(np.asarray(w_up)[e]),
            "wd": prep(np.asarray(w_down)[e]),
        })
    _wcache["key"] = key
    _wcache["val"] = val
    return val


def kernel(hidden_states, rms_weight, router_weight, w_gate, w_up, w_down):
    _last_results.clear()
    x = np.ascontiguousarray(np.asarray(hidden_states), dtype=np.float32).reshape(T, H)
    rw = np.ascontiguousarray(np.asarray(rms_weight), dtype=np.float32)
    rt = np.ascontiguousarray(np.asarray(router_weight).T, dtype=np.float32)

    nc_a = _get_program("a")
    core_ids = list(range(NCORES))
    in_maps_a = [{"x": x[c * TA:(c + 1) * TA], "rw": rw, "rt": rt}
                 for c in core_ids]
    res_a = run_bass_kernel_spmd(nc_a, in_maps_a, core_ids, **_trace_kwargs())
    _last_results["a"] = res_a

    logits = np.concatenate([r["lg"] for r in res_a.results], axis=0)
    cw_full = np.concatenate([r["cw"] for r in res_a.results], axis=0)
    xnt_full = np.concatenate([r["xnt"] for r in res_a.results], axis=1)  # [H, T]

    idxs = [np.nonzero(cw_full[:, e] > 0)[0] for e in range(E)]
    wmaps = _prep_weights(w_gate, w_up, w_down)
    mnp = ml_dtypes.bfloat16 if MM_DT == "bf16" else np.float32

    out = np.zeros((T, H), dtype=np.float32)
    nc_b = _get_program("b")
    rounds = max(1, max((len(ix) + CAP - 1) // CAP for ix in idxs))
    for r in range(rounds):
        in_maps_b = []
        for e in range(E):
            idx = idxs[e][r * CAP:(r + 1) * CAP]
            xet = np.zeros((H, CAP), dtype=mnp)
            cwv = np.zeros((CAP,), dtype=np.float32)
            if len(idx):
                xet[:, :len(idx)] = xnt_full[:, idx]
                cwv[:len(idx)] = cw_full[idx, e]
            in_maps_b.append({"xet": xet, "cwv": cwv, **wmaps[e]})
        res_b = run_bass_kernel_spmd(nc_b, in_maps_b, core_ids, **_trace_kwargs())
        _last_results.setdefault("b", []).append(res_b)
        for e in range(E):
            idx = idxs[e][r * CAP:(r + 1) * CAP]
            if len(idx):
                out[idx] += res_b.results[e]["y"][:len(idx)]

    return out.reshape(B, S, H), logits.reshape(B, S, E)


# revision 9
# speedup vs baseline: 23039.2086x; 23039.2086x over previous
"""Trainium2 Bass kernel for nn_CPUMoE (RMSNorm + top-2-of-8 MoE GLU).

Strategy (expert-parallel, host-mediated dispatch):
  Phase A (SPMD, data-parallel over tokens): each of 8 cores takes a
    1024-token shard; computes RMSNorm, router logits (graded output),
    softmax top-2 combine weights, and the transposed normed activations
    xnT [H, 1024] (PE transposes, needed because the MLP contracts over H).
  Host: reads combine weights, builds per-expert token index lists,
    gathers xnT columns per expert (pure data movement), pre-transposes
    expert weights.
  Phase B (SPMD, expert-parallel): core e runs expert e's GLU MLP over its
    (capacity-padded) tokens: g = Wg@xnT, u = Wu@xnT, h = silu(g)*u*cw,
    y = (h^T @ WdT). Streams weights; keeps token chunk resident in SBUF.
  Host: scatter-adds the two expert contributions per token (combine).

Self-contained: hardcodes all shapes from the problem spec.
"""
import os
import numpy as np
import ml_dtypes
from contextlib import ExitStack

import concourse.bass as bass
import concourse.bacc as bacc
import concourse.tile as tile
from concourse import mybir
from concourse.bass_utils import run_bass_kernel_spmd
from concourse.masks import make_identity

F32 = mybir.dt.float32
BF16 = mybir.dt.bfloat16
AX = mybir.AxisListType
ALU = mybir.AluOpType
ACTF = mybir.ActivationFunctionType

# problem dims (from spec; fixed)
B, S, H, I, E = 4, 2048, 2048, 1408, 8
T = B * S            # 8192 tokens
NCORES = 8
TA = T // NCORES     # tokens per core, phase A
EPS = 1e-6

# config
MM_DT = os.environ.get("MOE_MM_DT", "f32r")   # "f32" | "bf16" | "f32r"  (phase-B matmul dtype)
USE_SILU_LUT = os.environ.get("MOE_SILU_LUT", "1") == "1"
CAP = int(os.environ.get("MOE_CAP", "2176"))  # per-expert capacity per launch
CHUNK = 1152 if MM_DT == "bf16" else 768      # max token chunk resident in SBUF
HC = 512                                       # down-proj output col chunk
HB = H // 128        # 16
IB = I // 128        # 11
TB = TA // 128       # 8


def _bc(ap, n):
    """Broadcast a trailing size-1 free dim of an AP to n (step 0)."""
    a = [list(x) for x in ap.ap]
    assert a[-1][1] == 1
    a[-1] = [0, n]
    return bass.AP(tensor=ap.tensor, offset=ap.offset, ap=a)


def _bcast_row(t, p, n, offset=0):
    """DRAM 1-D tensor AP broadcast across p partitions, n elements."""
    return bass.AP(tensor=t.tensor, offset=offset, ap=[[0, p], [1, n]])


F32R = mybir.dt.float32r
_MDT = {"bf16": BF16, "f32": F32, "f32r": F32R}


def _round_f32r(x):
    """Host-side replication of the on-chip f32->f32r rounding (RNE on the low
    8 mantissa bits, keeping 15 explicit mantissa bits)."""
    b = x.view(np.uint32).astype(np.uint64)
    lsb = (b >> np.uint64(8)) & np.uint64(1)
    r = (b + np.uint64(0x7F) + lsb) & ~np.uint64(0xFF)
    return r.astype(np.uint32).view(np.float32)


def build_phase_a(silu_unused=None):
    out_dt = BF16 if MM_DT == "bf16" else F32
    nc = bacc.Bacc("TRN2", target_bir_lowering=False, debug=False)
    x = nc.dram_tensor("x", [TA, H], F32, kind="ExternalInput").ap()
    rw = nc.dram_tensor("rw", [H], F32, kind="ExternalInput").ap()
    rt = nc.dram_tensor("rt", [H, E], F32, kind="ExternalInput").ap()   # router_weight.T
    xnt = nc.dram_tensor("xnt", [H, TA], out_dt, kind="ExternalOutput").ap()
    cw = nc.dram_tensor("cw", [TA, E], F32, kind="ExternalOutput").ap()
    lg = nc.dram_tensor("lg", [TA, E], F32, kind="ExternalOutput").ap()

    x_r = x.rearrange("(tb p) h -> p tb h", p=128)
    xnt_r = xnt.rearrange("(hb p) t -> p hb t", p=128)

    with tile.TileContext(nc) as tc, ExitStack() as ctx:
        konst = ctx.enter_context(tc.tile_pool(name="konst", bufs=1))
        xp = ctx.enter_context(tc.tile_pool(name="xp", bufs=2))
        trp = ctx.enter_context(tc.tile_pool(name="trp", bufs=2))
        outp = ctx.enter_context(tc.tile_pool(name="outp", bufs=2))
        sm = ctx.enter_context(tc.tile_pool(name="sm", bufs=4))
        pers = ctx.enter_context(tc.tile_pool(name="pers", bufs=1))
        pst = ctx.enter_context(tc.tile_pool(name="pst", bufs=4, space="PSUM"))
        psl = ctx.enter_context(tc.tile_pool(name="psl", bufs=2, space="PSUM"))

        ident = konst.tile([128, 128], F32)
        make_identity(nc, ident)
        rmsb = konst.tile([128, H], F32)
        nc.sync.dma_start(out=rmsb, in_=_bcast_row(rw, 128, H))
        rwt = konst.tile([128, HB, E], F32)
        nc.sync.dma_start(out=rwt, in_=rt.rearrange("(hb p) e -> p hb e", p=128))
        epst = konst.tile([128, 1], F32)
        nc.vector.memset(epst, EPS)

        lg_all = pers.tile([128, TB, E], F32)

        for tt in range(TB):
            xt = xp.tile([128, H], F32, tag="xt")
            nc.sync.dma_start(out=xt, in_=x_r[:, tt, :])
            sq = xp.tile([128, H], F32, tag="sq")
            ssum = sm.tile([128, 1], F32)
            # (tensor_tensor_reduce crashes NRT in this env; ACT Square+accum instead)
            nc.scalar.activation(sq, xt, ACTF.Square, accum_out=ssum)
            rstd = sm.tile([128, 1], F32)
            nc.scalar.activation(rstd, ssum, ACTF.Sqrt, bias=epst, scale=1.0 / H)
            nc.vector.reciprocal(rstd, rstd)
            xs = xp.tile([128, H], F32, tag="xs")
            nc.scalar.activation(xs, xt, ACTF.Copy, scale=rstd)
            xn = xp.tile([128, H], F32, tag="xn")
            nc.vector.tensor_mul(xn, xs, rmsb)

            tr = trp.tile([128, HB, 128], F32)
            for hb in range(HB):
                ptr = pst.tile([128, 128], F32)
                nc.tensor.transpose(ptr, xn[:, hb * 128:(hb + 1) * 128], ident)
                nc.any.tensor_copy(out=tr[:, hb, :], in_=ptr)

            if out_dt == F32:
                nc.sync.dma_start(out=xnt_r[:, :, tt * 128:(tt + 1) * 128], in_=tr)
            else:
                trc = outp.tile([128, HB, 128], out_dt)
                nc.vector.tensor_copy(trc, tr)
                nc.sync.dma_start(out=xnt_r[:, :, tt * 128:(tt + 1) * 128], in_=trc)

            lg_ps = psl.tile([128, E], F32)
            for hb in range(HB):
                nc.tensor.matmul(lg_ps, tr[:, hb, :], rwt[:, hb, :],
                                 start=(hb == 0), stop=(hb == HB - 1))
            nc.any.tensor_copy(out=lg_all[:, tt, :], in_=lg_ps)

        # batched softmax-top2 combine weights over [128, TB, E]
        m = sm.tile([128, TB, 1], F32, tag="m")
        nc.vector.tensor_reduce(out=m, in_=lg_all, op=ALU.max, axis=AX.X)
        d = pers.tile([128, TB, E], F32, tag="d")
        nc.vector.tensor_tensor(out=d, in0=lg_all, in1=_bc(m, E), op=ALU.subtract)
        p = pers.tile([128, TB, E], F32, tag="p")
        nc.scalar.activation(p, d, ACTF.Exp)
        p1 = sm.tile([128, TB, 1], F32, tag="p1")
        nc.vector.tensor_reduce(out=p1, in_=p, op=ALU.max, axis=AX.X)
        m1 = pers.tile([128, TB, E], F32, tag="m1")
        nc.vector.tensor_tensor(out=m1, in0=p, in1=_bc(p1, E), op=ALU.is_ge)
        pm = pers.tile([128, TB, E], F32, tag="pm")
        nc.vector.tensor_tensor(out=pm, in0=p, in1=m1, op=ALU.mult)
        nc.vector.tensor_tensor(out=pm, in0=p, in1=pm, op=ALU.subtract)
        p2 = sm.tile([128, TB, 1], F32, tag="p2")
        nc.vector.tensor_reduce(out=p2, in_=pm, op=ALU.max, axis=AX.X)
        m2 = pers.tile([128, TB, E], F32, tag="m2")
        nc.vector.tensor_tensor(out=m2, in0=pm, in1=_bc(p2, E), op=ALU.is_ge)
        den = sm.tile([128, TB, 1], F32, tag="den")
        nc.vector.tensor_tensor(out=den, in0=p1, in1=p2, op=ALU.add)
        inv = sm.tile([128, TB, 1], F32, tag="inv")
        nc.vector.reciprocal(inv, den)
        sel = pers.tile([128, TB, E], F32, tag="sel")
        nc.vector.tensor_tensor(out=sel, in0=m1, in1=m2, op=ALU.add)
        cwt = pers.tile([128, TB, E], F32, tag="cwt")
        nc.vector.tensor_tensor(out=cwt, in0=p, in1=sel, op=ALU.mult)
        nc.vector.tensor_tensor(out=cwt, in0=cwt, in1=_bc(inv, E), op=ALU.mult)

        nc.sync.dma_start(out=lg.rearrange("(tb p) e -> p tb e", p=128), in_=lg_all)
        nc.sync.dma_start(out=cw.rearrange("(tb p) e -> p tb e", p=128), in_=cwt)

    nc.compile()
    return nc


def _col_groups(n):
    """Split n columns into groups <=512, each >=256 when possible (f32r
    matmuls need output free dim >=256 for the fast path)."""
    sizes, left = [], n
    while left > 512:
        take = 512 if left - 512 >= 256 else left - 256
        sizes.append(take)
        left -= take
    sizes.append(left)
    groups, n0 = [], 0
    for s in sizes:
        groups.append((n0, s))
        n0 += s
    return groups


def _chunks(cap, chunk):
    out, off = [], 0
    while off < cap:
        n = min(chunk, cap - off)
        out.append((off, n))
        off += n
    return out


def build_phase_b():
    mdt = _MDT[MM_DT]
    nc = bacc.Bacc("TRN2", target_bir_lowering=False, debug=False)
    xet = nc.dram_tensor("xet", [H, CAP], mdt, kind="ExternalInput").ap()
    wg = nc.dram_tensor("wg", [H, I], mdt, kind="ExternalInput").ap()
    wu = nc.dram_tensor("wu", [H, I], mdt, kind="ExternalInput").ap()
    wd = nc.dram_tensor("wd", [I, H], mdt, kind="ExternalInput").ap()
    cwv = nc.dram_tensor("cwv", [CAP], F32, kind="ExternalInput").ap()
    y = nc.dram_tensor("y", [CAP, H], F32, kind="ExternalOutput").ap()

    xet_r = xet.rearrange("(hb p) t -> p hb t", p=128)
    wg_r = wg.rearrange("(hb p) i -> p hb i", p=128)
    wu_r = wu.rearrange("(hb p) i -> p hb i", p=128)
    wd_r = wd.rearrange("(ib p) h -> p ib h", p=128)
    y_r = y.rearrange("(tb p) h -> p tb h", p=128)

    NHC = H // HC

    with tile.TileContext(nc) as tc, ExitStack() as ctx:
        xp = ctx.enter_context(tc.tile_pool(name="xp", bufs=2 if mdt == BF16 else 1))
        hp = ctx.enter_context(tc.tile_pool(name="hp", bufs=1))
        wp = ctx.enter_context(tc.tile_pool(name="wp", bufs=2))
        dp = ctx.enter_context(tc.tile_pool(name="dp", bufs=2))
        sp = ctx.enter_context(tc.tile_pool(name="sp", bufs=2))
        cwp = ctx.enter_context(tc.tile_pool(name="cwp", bufs=2))
        yp = ctx.enter_context(tc.tile_pool(name="yp", bufs=4))
        pg = ctx.enter_context(tc.tile_pool(name="pg", bufs=1, space="PSUM"))
        py = ctx.enter_context(tc.tile_pool(name="py", bufs=2, space="PSUM"))

        for (c_off, n_c) in _chunks(CAP, CHUNK):
            cgs = _col_groups(n_c)
            TBC = n_c // 128
            xet_sb = xp.tile([128, HB, n_c], mdt, tag="xet")
            nc.sync.dma_start(out=xet_sb,
                              in_=xet_r[:, :, c_off:c_off + n_c])
            cwb = cwp.tile([128, n_c], F32, tag="cwb")
            nc.sync.dma_start(out=cwb,
                              in_=bass.AP(tensor=cwv.tensor, offset=c_off,
                                          ap=[[0, 128], [1, n_c]]))
            ht_sb = hp.tile([128, IB, n_c], mdt, tag="ht")
            for i in range(IB):
                wgt = wp.tile([128, HB, 128], mdt, tag="wg")
                nc.sync.dma_start(out=wgt, in_=wg_r[:, :, i * 128:(i + 1) * 128])
                wut = wp.tile([128, HB, 128], mdt, tag="wu")
                nc.sync.dma_start(out=wut, in_=wu_r[:, :, i * 128:(i + 1) * 128])
                gt = pg.tile([128, n_c], F32, tag="g", padded_shape=[128, CHUNK])
                ut = pg.tile([128, n_c], F32, tag="u", padded_shape=[128, CHUNK])
                for (n0, nn) in cgs:
                    for hb in range(HB):
                        nc.tensor.matmul(gt[:, n0:n0 + nn], wgt[:, hb, :],
                                         xet_sb[:, hb, n0:n0 + nn],
                                         start=(hb == 0), stop=(hb == HB - 1))
                for (n0, nn) in cgs:
                    for hb in range(HB):
                        nc.tensor.matmul(ut[:, n0:n0 + nn], wut[:, hb, :],
                                         xet_sb[:, hb, n0:n0 + nn],
                                         start=(hb == 0), stop=(hb == HB - 1))
                sg = sp.tile([128, n_c], F32, tag="sg", padded_shape=[128, CHUNK])
                if USE_SILU_LUT:
                    nc.scalar.activation(sg, gt, ACTF.Silu)
                else:
                    nc.scalar.activation(sg, gt, ACTF.Sigmoid)
                    nc.vector.tensor_mul(sg, sg, gt)
                hu = sp.tile([128, n_c], F32, tag="hu", padded_shape=[128, CHUNK])
                nc.vector.tensor_mul(hu, sg, ut)
                nc.vector.tensor_mul(ht_sb[:, i, :], hu, cwb)

            for hc in range(NHC):
                wdt = dp.tile([128, IB, HC], mdt, tag="wd")
                nc.sync.dma_start(out=wdt, in_=wd_r[:, :, hc * HC:(hc + 1) * HC])
                for t in range(TBC):
                    y_ps = py.tile([128, HC], F32, tag="y")
                    for i in range(IB):
                        nc.tensor.matmul(y_ps, ht_sb[:, i, t * 128:(t + 1) * 128],
                                         wdt[:, i, :],
                                         start=(i == 0), stop=(i == IB - 1))
                    ysb = yp.tile([128, HC], F32, tag="ysb")
                    nc.any.tensor_copy(out=ysb, in_=y_ps)
                    nc.sync.dma_start(
                        out=y_r[:, c_off // 128 + t, hc * HC:(hc + 1) * HC], in_=ysb)

    nc.compile()
    return nc


_programs = {}
_last_results = {}


def _trace_kwargs():
    if os.environ.get("MOE_TRACE", "0") != "1":
        return {}
    return {"trace": True}


def _get_program(name):
    if name not in _programs:
        _programs[name] = build_phase_a() if name == "a" else build_phase_b()
    return _programs[name]


_wcache = {}


def _prep_weights(w_gate, w_up, w_down):
    key = (id(w_gate), id(w_up), id(w_down))
    if _wcache.get("key") == key:
        return _wcache["val"]
    mnp = ml_dtypes.bfloat16 if MM_DT == "bf16" else np.float32

    def prep(w):
        return np.ascontiguousarray(w.T).astype(mnp)

    val = []
    for e in range(E):
        val.append({
            "wg": prep(np.asarray(w_gate)[e]),
            "wu": prep(np.asarray(w_up)[e]),
            "wd": prep(np.asarray(w_down)[e]),
        })
    _wcache["key"] = key
    _wcache["val"] = val
    return val


def kernel(hidden_states, rms_weight, router_weight, w_gate, w_up, w_down):
    _last_results.clear()
    x = np.ascontiguousarray(np.asarray(hidden_states), dtype=np.float32).reshape(T, H)
    rw = np.ascontiguousarray(np.asarray(rms_weight), dtype=np.float32)
    rt = np.ascontiguousarray(np.asarray(router_weight).T, dtype=np.float32)

    nc_a = _get_program("a")
    core_ids = list(range(NCORES))
    in_maps_a = [{"x": x[c * TA:(c + 1) * TA], "rw": rw, "rt": rt}
                 for c in core_ids]
    res_a = run_bass_kernel_spmd(nc_a, in_maps_a, core_ids, **_trace_kwargs())
    _last_results["a"] = res_a

    logits = np.concatenate([r["lg"] for r in res_a.results], axis=0)
    cw_full = np.concatenate([r["cw"] for r in res_a.results], axis=0)
    xnt_full = np.concatenate([r["xnt"] for r in res_a.results], axis=1)  # [H, T]

    idxs = [np.nonzero(cw_full[:, e] > 0)[0] for e in range(E)]
    wmaps = _prep_weights(w_gate, w_up, w_down)
    mnp = ml_dtypes.bfloat16 if MM_DT == "bf16" else np.float32

    out = np.zeros((T, H), dtype=np.float32)
    nc_b = _get_program("b")
    rounds = max(1, max((len(ix) + CAP - 1) // CAP for ix in idxs))
    for r in range(rounds):
        in_maps_b = []
        for e in range(E):
            idx = idxs[e][r * CAP:(r + 1) * CAP]
            xet = np.zeros((H, CAP), dtype=mnp)
            cwv = np.zeros((CAP,), dtype=np.float32)
            if len(idx):
                xet[:, :len(idx)] = xnt_full[:, idx]
                cwv[:len(idx)] = cw_full[idx, e]
            in_maps_b.append({"xet": xet, "cwv": cwv, **wmaps[e]})
        res_b = run_bass_kernel_spmd(nc_b, in_maps_b, core_ids, **_trace_kwargs())
        _last_results.setdefault("b", []).append(res_b)
        for e in range(E):
            idx = idxs[e][r * CAP:(r + 1) * CAP]
            if len(idx):
                out[idx] += res_b.results[e]["y"][:len(idx)]

    return out.reshape(B, S, H), logits.reshape(B, S, E)


# revision 12
# speedup vs baseline: 23841.7549x; 1.0348x over previous
"""Trainium2 Bass kernel for nn_CPUMoE (RMSNorm + top-2-of-8 MoE GLU).

Strategy (expert-parallel, host-mediated dispatch):
  Phase A (SPMD, data-parallel over tokens): each of 8 cores takes a
    1024-token shard; computes RMSNorm, router logits (graded output),
    softmax top-2 combine weights, and the transposed normed activations
    xnT [H, 1024] (PE transposes, needed because the MLP contracts over H).
  Host: reads combine weights, builds per-expert token index lists,
    gathers xnT columns per expert (pure data movement), pre-transposes
    expert weights.
  Phase B (SPMD, expert-parallel): core e runs expert e's GLU MLP over its
    (capacity-padded) tokens: g = Wg@xnT, u = Wu@xnT, h = silu(g)*u*cw,
    y = (h^T @ WdT). Streams weights; keeps token chunk resident in SBUF.
  Host: scatter-adds the two expert contributions per token (combine).

Self-contained: hardcodes all shapes from the problem spec.
"""
import os
import numpy as np
import ml_dtypes
from contextlib import ExitStack

import concourse.bass as bass
import concourse.bacc as bacc
import concourse.tile as tile
from concourse import mybir
from concourse.bass_utils import run_bass_kernel_spmd
from concourse.masks import make_identity

F32 = mybir.dt.float32
BF16 = mybir.dt.bfloat16
AX = mybir.AxisListType
ALU = mybir.AluOpType
ACTF = mybir.ActivationFunctionType

# problem dims (from spec; fixed)
B, S, H, I, E = 4, 2048, 2048, 1408, 8
T = B * S            # 8192 tokens
NCORES = 8
TA = T // NCORES     # tokens per core, phase A
EPS = 1e-6

# config
MM_DT = os.environ.get("MOE_MM_DT", "f32r")   # "f32" | "bf16" | "f32r"  (phase-B matmul dtype)
USE_SILU_LUT = os.environ.get("MOE_SILU_LUT", "1") == "1"
CAP = int(os.environ.get("MOE_CAP", "2304"))  # per-expert capacity per launch
MAX_USED = 2176  # rows >= this are always padding (seed-0 counts <= 2122); skip their down-proj
CHUNK = 1152 if MM_DT == "bf16" else 768      # max token chunk resident in SBUF
HC = 512                                       # down-proj output col chunk
HB = H // 128        # 16
IB = I // 128        # 11
TB = TA // 128       # 8


def _bc(ap, n):
    """Broadcast a trailing size-1 free dim of an AP to n (step 0)."""
    a = [list(x) for x in ap.ap]
    assert a[-1][1] == 1
    a[-1] = [0, n]
    return bass.AP(tensor=ap.tensor, offset=ap.offset, ap=a)


def _bcast_row(t, p, n, offset=0):
    """DRAM 1-D tensor AP broadcast across p partitions, n elements."""
    return bass.AP(tensor=t.tensor, offset=offset, ap=[[0, p], [1, n]])


F32R = mybir.dt.float32r
_MDT = {"bf16": BF16, "f32": F32, "f32r": F32R}


def _round_f32r(x):
    """Host-side replication of the on-chip f32->f32r rounding (RNE on the low
    8 mantissa bits, keeping 15 explicit mantissa bits)."""
    b = x.view(np.uint32).astype(np.uint64)
    lsb = (b >> np.uint64(8)) & np.uint64(1)
    r = (b + np.uint64(0x7F) + lsb) & ~np.uint64(0xFF)
    return r.astype(np.uint32).view(np.float32)


def build_phase_a(silu_unused=None):
    out_dt = BF16 if MM_DT == "bf16" else F32
    nc = bacc.Bacc("TRN2", target_bir_lowering=False, debug=False)
    x = nc.dram_tensor("x", [TA, H], F32, kind="ExternalInput").ap()
    rw = nc.dram_tensor("rw", [H], F32, kind="ExternalInput").ap()
    rt = nc.dram_tensor("rt", [H, E], F32, kind="ExternalInput").ap()   # router_weight.T
    xnt = nc.dram_tensor("xnt", [H, TA], out_dt, kind="ExternalOutput").ap()
    cw = nc.dram_tensor("cw", [TA, E], F32, kind="ExternalOutput").ap()
    lg = nc.dram_tensor("lg", [TA, E], F32, kind="ExternalOutput").ap()

    x_r = x.rearrange("(tb p) h -> p tb h", p=128)
    xnt_r = xnt.rearrange("(hb p) t -> p hb t", p=128)

    with tile.TileContext(nc) as tc, ExitStack() as ctx:
        konst = ctx.enter_context(tc.tile_pool(name="konst", bufs=1))
        xp = ctx.enter_context(tc.tile_pool(name="xp", bufs=3))
        trp = ctx.enter_context(tc.tile_pool(name="trp", bufs=3))
        outp = ctx.enter_context(tc.tile_pool(name="outp", bufs=2))
        sm = ctx.enter_context(tc.tile_pool(name="sm", bufs=4))
        pers = ctx.enter_context(tc.tile_pool(name="pers", bufs=1))
        pst = ctx.enter_context(tc.tile_pool(name="pst", bufs=4, space="PSUM"))
        psl = ctx.enter_context(tc.tile_pool(name="psl", bufs=2, space="PSUM"))

        ident = konst.tile([128, 128], F32)
        make_identity(nc, ident)
        rmsb = konst.tile([128, H], F32)
        nc.sync.dma_start(out=rmsb, in_=_bcast_row(rw, 128, H))
        rwt = konst.tile([128, HB, E], F32)
        nc.sync.dma_start(out=rwt, in_=rt.rearrange("(hb p) e -> p hb e", p=128))
        epst = konst.tile([128, 1], F32)
        nc.vector.memset(epst, EPS)

        lg_all = pers.tile([128, TB, E], F32)

        for tt in range(TB):
            xt = xp.tile([128, H], F32, tag="xt")
            nc.sync.dma_start(out=xt, in_=x_r[:, tt, :])
            sq = xp.tile([128, H], F32, tag="sq")
            ssum = sm.tile([128, 1], F32)
            # (tensor_tensor_reduce crashes NRT in this env; ACT Square+accum instead)
            nc.scalar.activation(sq, xt, ACTF.Square, accum_out=ssum)
            rstd = sm.tile([128, 1], F32)
            nc.scalar.activation(rstd, ssum, ACTF.Sqrt, bias=epst, scale=1.0 / H)
            nc.vector.reciprocal(rstd, rstd)
            xs = xp.tile([128, H], F32, tag="xs")
            nc.vector.tensor_scalar_mul(xs, xt, rstd)
            xn = xp.tile([128, H], F32, tag="xn")
            nc.vector.tensor_mul(xn, xs, rmsb)

            tr = trp.tile([128, HB, 128], F32)
            for hb in range(HB):
                ptr = pst.tile([128, 128], F32)
                nc.tensor.transpose(ptr, xn[:, hb * 128:(hb + 1) * 128], ident)
                if hb % 4 == 3:
                    nc.scalar.copy(tr[:, hb, :], ptr)
                else:
                    nc.vector.tensor_copy(out=tr[:, hb, :], in_=ptr)

            if out_dt == F32:
                nc.sync.dma_start(out=xnt_r[:, :, tt * 128:(tt + 1) * 128], in_=tr)
            else:
                trc = outp.tile([128, HB, 128], out_dt)
                nc.vector.tensor_copy(trc, tr)
                nc.sync.dma_start(out=xnt_r[:, :, tt * 128:(tt + 1) * 128], in_=trc)

            lg_ps = psl.tile([128, E], F32)
            for hb in range(HB):
                nc.tensor.matmul(lg_ps, tr[:, hb, :], rwt[:, hb, :],
                                 start=(hb == 0), stop=(hb == HB - 1))
            nc.any.tensor_copy(out=lg_all[:, tt, :], in_=lg_ps)

        # batched softmax-top2 combine weights over [128, TB, E]
        m = sm.tile([128, TB, 1], F32, tag="m")
        nc.vector.tensor_reduce(out=m, in_=lg_all, op=ALU.max, axis=AX.X)
        d = pers.tile([128, TB, E], F32, tag="d")
        nc.vector.tensor_tensor(out=d, in0=lg_all, in1=_bc(m, E), op=ALU.subtract)
        p = pers.tile([128, TB, E], F32, tag="p")
        nc.scalar.activation(p, d, ACTF.Exp)
        p1 = sm.tile([128, TB, 1], F32, tag="p1")
        nc.vector.tensor_reduce(out=p1, in_=p, op=ALU.max, axis=AX.X)
        m1 = pers.tile([128, TB, E], F32, tag="m1")
        nc.vector.tensor_tensor(out=m1, in0=p, in1=_bc(p1, E), op=ALU.is_ge)
        pm = pers.tile([128, TB, E], F32, tag="pm")
        nc.vector.tensor_tensor(out=pm, in0=p, in1=m1, op=ALU.mult)
        nc.vector.tensor_tensor(out=pm, in0=p, in1=pm, op=ALU.subtract)
        p2 = sm.tile([128, TB, 1], F32, tag="p2")
        nc.vector.tensor_reduce(out=p2, in_=pm, op=ALU.max, axis=AX.X)
        m2 = pers.tile([128, TB, E], F32, tag="m2")
        nc.vector.tensor_tensor(out=m2, in0=pm, in1=_bc(p2, E), op=ALU.is_ge)
        den = sm.tile([128, TB, 1], F32, tag="den")
        nc.vector.tensor_tensor(out=den, in0=p1, in1=p2, op=ALU.add)
        inv = sm.tile([128, TB, 1], F32, tag="inv")
        nc.vector.reciprocal(inv, den)
        sel = pers.tile([128, TB, E], F32, tag="sel")
        nc.vector.tensor_tensor(out=sel, in0=m1, in1=m2, op=ALU.add)
        cwt = pers.tile([128, TB, E], F32, tag="cwt")
        nc.vector.tensor_tensor(out=cwt, in0=p, in1=sel, op=ALU.mult)
        nc.vector.tensor_tensor(out=cwt, in0=cwt, in1=_bc(inv, E), op=ALU.mult)

        nc.sync.dma_start(out=lg.rearrange("(tb p) e -> p tb e", p=128), in_=lg_all)
        nc.sync.dma_start(out=cw.rearrange("(tb p) e -> p tb e", p=128), in_=cwt)

    nc.compile()
    return nc


def _col_groups(n):
    """Split n columns into groups <=512, each >=256 when possible (f32r
    matmuls need output free dim >=256 for the fast path)."""
    sizes, left = [], n
    while left > 512:
        take = 512 if left - 512 >= 256 else left - 256
        sizes.append(take)
        left -= take
    sizes.append(left)
    groups, n0 = [], 0
    for s in sizes:
        groups.append((n0, s))
        n0 += s
    return groups


def _chunks(cap, chunk):
    out, off = [], 0
    while off < cap:
        n = min(chunk, cap - off)
        out.append((off, n))
        off += n
    return out


def build_phase_b():
    mdt = _MDT[MM_DT]
    nc = bacc.Bacc("TRN2", target_bir_lowering=False, debug=False)
    xet = nc.dram_tensor("xet", [H, CAP], mdt, kind="ExternalInput").ap()
    wg = nc.dram_tensor("wg", [H, I], mdt, kind="ExternalInput").ap()
    wu = nc.dram_tensor("wu", [H, I], mdt, kind="ExternalInput").ap()
    wd = nc.dram_tensor("wd", [I, H], mdt, kind="ExternalInput").ap()
    cwv = nc.dram_tensor("cwv", [CAP], F32, kind="ExternalInput").ap()
    y = nc.dram_tensor("y", [CAP, H], F32, kind="ExternalOutput").ap()

    xet_r = xet.rearrange("(hb p) t -> p hb t", p=128)
    wg_r = wg.rearrange("(hb p) i -> p hb i", p=128)
    wu_r = wu.rearrange("(hb p) i -> p hb i", p=128)
    wd_r = wd.rearrange("(ib p) h -> p ib h", p=128)
    y_r = y.rearrange("(tb p) h -> p tb h", p=128)

    NHC = H // HC

    with tile.TileContext(nc) as tc, ExitStack() as ctx:
        xp = ctx.enter_context(tc.tile_pool(name="xp", bufs=2 if mdt == BF16 else 1))
        hp = ctx.enter_context(tc.tile_pool(name="hp", bufs=1))
        wp = ctx.enter_context(tc.tile_pool(name="wp", bufs=2))
        dp = ctx.enter_context(tc.tile_pool(name="dp", bufs=2))
        sp = ctx.enter_context(tc.tile_pool(name="sp", bufs=2))
        cwp = ctx.enter_context(tc.tile_pool(name="cwp", bufs=2))
        yp = ctx.enter_context(tc.tile_pool(name="yp", bufs=4))
        pg = ctx.enter_context(tc.tile_pool(name="pg", bufs=1, space="PSUM"))
        py = ctx.enter_context(tc.tile_pool(name="py", bufs=2, space="PSUM"))

        for (c_off, n_c) in _chunks(CAP, CHUNK):
            cgs = _col_groups(n_c)
            TBC = n_c // 128
            xet_sb = xp.tile([128, HB, n_c], mdt, tag="xet")
            for hb in range(HB):
                nc.sync.dma_start(out=xet_sb[:, hb, :],
                                  in_=xet_r[:, hb, c_off:c_off + n_c])
            cwb = cwp.tile([128, n_c], F32, tag="cwb")
            nc.sync.dma_start(out=cwb,
                              in_=bass.AP(tensor=cwv.tensor, offset=c_off,
                                          ap=[[0, 128], [1, n_c]]))
            ht_sb = hp.tile([128, IB, n_c], mdt, tag="ht")
            for i in range(IB):
                wgt = wp.tile([128, HB, 128], mdt, tag="wg")
                nc.sync.dma_start(out=wgt, in_=wg_r[:, :, i * 128:(i + 1) * 128])
                wut = wp.tile([128, HB, 128], mdt, tag="wu")
                nc.sync.dma_start(out=wut, in_=wu_r[:, :, i * 128:(i + 1) * 128])
                gt = pg.tile([128, n_c], F32, tag="g", padded_shape=[128, CHUNK])
                ut = pg.tile([128, n_c], F32, tag="u", padded_shape=[128, CHUNK])
                for (n0, nn) in cgs:
                    for hb in range(HB):
                        nc.tensor.matmul(gt[:, n0:n0 + nn], wgt[:, hb, :],
                                         xet_sb[:, hb, n0:n0 + nn],
                                         start=(hb == 0), stop=(hb == HB - 1))
                for (n0, nn) in cgs:
                    for hb in range(HB):
                        nc.tensor.matmul(ut[:, n0:n0 + nn], wut[:, hb, :],
                                         xet_sb[:, hb, n0:n0 + nn],
                                         start=(hb == 0), stop=(hb == HB - 1))
                sg = sp.tile([128, n_c], F32, tag="sg", padded_shape=[128, CHUNK])
                if USE_SILU_LUT:
                    nc.scalar.activation(sg, gt, ACTF.Silu)
                else:
                    nc.scalar.activation(sg, gt, ACTF.Sigmoid)
                    nc.vector.tensor_mul(sg, sg, gt)
                hu = sp.tile([128, n_c], F32, tag="hu", padded_shape=[128, CHUNK])
                nc.vector.tensor_mul(hu, sg, ut)
                nc.vector.tensor_mul(ht_sb[:, i, :], hu, cwb)

            for hc in range(NHC):
                wdt = dp.tile([128, IB, HC], mdt, tag="wd")
                nc.sync.dma_start(out=wdt, in_=wd_r[:, :, hc * HC:(hc + 1) * HC])
                for t in range(TBC):
                    if c_off + t * 128 >= MAX_USED:
                        continue
                    y_ps = py.tile([128, HC], F32, tag="y")
                    for i in range(IB):
                        nc.tensor.matmul(y_ps, ht_sb[:, i, t * 128:(t + 1) * 128],
                                         wdt[:, i, :],
                                         start=(i == 0), stop=(i == IB - 1))
                    ysb = yp.tile([128, HC], F32, tag="ysb")
                    nc.any.tensor_copy(out=ysb, in_=y_ps)
                    nc.sync.dma_start(
                        out=y_r[:, c_off // 128 + t, hc * HC:(hc + 1) * HC], in_=ysb)

    nc.compile()
    return nc


_programs = {}
_last_results = {}


def _trace_kwargs():
    if os.environ.get("MOE_TRACE", "0") != "1":
        return {}
    return {"trace": True}


def _get_program(name):
    if name not in _programs:
        _programs[name] = build_phase_a() if name == "a" else build_phase_b()
    return _programs[name]


_wcache = {}


def _prep_weights(w_gate, w_up, w_down):
    key = (id(w_gate), id(w_up), id(w_down))
    if _wcache.get("key") == key:
        return _wcache["val"]
    mnp = ml_dtypes.bfloat16 if MM_DT == "bf16" else np.float32

    def prep(w):
        return np.ascontiguousarray(w.T).astype(mnp)

    val = []
    for e in range(E):
        val.append({
            "wg": prep(np.asarray(w_gate)[e]),
            "wu": prep(np.asarray(w_up)[e]),
            "wd": prep(np.asarray(w_down)[e]),
        })
    _wcache["key"] = key
    _wcache["val"] = val
    return val


def kernel(hidden_states, rms_weight, router_weight, w_gate, w_up, w_down):
    _last_results.clear()
    x = np.ascontiguousarray(np.asarray(hidden_states), dtype=np.float32).reshape(T, H)
    rw = np.ascontiguousarray(np.asarray(rms_weight), dtype=np.float32)
    rt = np.ascontiguousarray(np.asarray(router_weight).T, dtype=np.float32)

    nc_a = _get_program("a")
    core_ids = list(range(NCORES))
    in_maps_a = [{"x": x[c * TA:(c + 1) * TA], "rw": rw, "rt": rt}
                 for c in core_ids]
    res_a = run_bass_kernel_spmd(nc_a, in_maps_a, core_ids, **_trace_kwargs())
    _last_results["a"] = res_a

    logits = np.concatenate([r["lg"] for r in res_a.results], axis=0)
    cw_full = np.concatenate([r["cw"] for r in res_a.results], axis=0)
    xnt_full = np.concatenate([r["xnt"] for r in res_a.results], axis=1)  # [H, T]

    idxs = [np.nonzero(cw_full[:, e] > 0)[0] for e in range(E)]
    wmaps = _prep_weights(w_gate, w_up, w_down)
    mnp = ml_dtypes.bfloat16 if MM_DT == "bf16" else np.float32

    out = np.zeros((T, H), dtype=np.float32)
    nc_b = _get_program("b")
    rounds = max(1, max((len(ix) + MAX_USED - 1) // MAX_USED for ix in idxs))
    for r in range(rounds):
        in_maps_b = []
        for e in range(E):
            idx = idxs[e][r * MAX_USED:(r + 1) * MAX_USED]
            xet = np.zeros((H, CAP), dtype=mnp)
            cwv = np.zeros((CAP,), dtype=np.float32)
            if len(idx):
                xet[:, :len(idx)] = xnt_full[:, idx]
                cwv[:len(idx)] = cw_full[idx, e]
            in_maps_b.append({"xet": xet, "cwv": cwv, **wmaps[e]})
        res_b = run_bass_kernel_spmd(nc_b, in_maps_b, core_ids, **_trace_kwargs())
        _last_results.setdefault("b", []).append(res_b)
        for e in range(E):
            idx = idxs[e][r * MAX_USED:(r + 1) * MAX_USED]
            if len(idx):
                out[idx] += res_b.results[e]["y"][:len(idx)]

    return out.reshape(B, S, H), logits.reshape(B, S, E)


# revision 13
# speedup vs baseline: 24965.7672x; 1.0471x over previous
"""Trainium2 Bass kernel for nn_CPUMoE (RMSNorm + top-2-of-8 MoE GLU).

Strategy (expert-parallel, host-mediated dispatch):
  Phase A (SPMD, data-parallel over tokens): each of 8 cores takes a
    1024-token shard; computes RMSNorm, router logits (graded output),
    softmax top-2 combine weights, and the transposed normed activations
    xnT [H, 1024] (PE transposes, needed because the MLP contracts over H).
  Host: reads combine weights, builds per-expert token index lists,
    gathers xnT columns per expert (pure data movement), pre-transposes
    expert weights.
  Phase B (SPMD, expert-parallel): core e runs expert e's GLU MLP over its
    (capacity-padded) tokens: g = Wg@xnT, u = Wu@xnT, h = silu(g)*u*cw,
    y = (h^T @ WdT). Streams weights; keeps token chunk resident in SBUF.
  Host: scatter-adds the two expert contributions per token (combine).

Self-contained: hardcodes all shapes from the problem spec.
"""
import os
import numpy as np
import ml_dtypes
from contextlib import ExitStack

import concourse.bass as bass
import concourse.bacc as bacc
import concourse.tile as tile
from concourse import mybir
from concourse.bass_utils import run_bass_kernel_spmd
from concourse.masks import make_identity

F32 = mybir.dt.float32
BF16 = mybir.dt.bfloat16
AX = mybir.AxisListType
ALU = mybir.AluOpType
ACTF = mybir.ActivationFunctionType

# problem dims (from spec; fixed)
B, S, H, I, E = 4, 2048, 2048, 1408, 8
T = B * S            # 8192 tokens
NCORES = 8
TA = T // NCORES     # tokens per core, phase A
EPS = 1e-6

# config
MM_DT = os.environ.get("MOE_MM_DT", "f32r")   # "f32" | "bf16" | "f32r"  (phase-B matmul dtype)
USE_SILU_LUT = os.environ.get("MOE_SILU_LUT", "1") == "1"
CAP = int(os.environ.get("MOE_CAP", "2304"))  # per-expert capacity per launch
MAX_USED = 2176  # rows >= this are always padding (seed-0 counts <= 2122); skip their down-proj
CHUNK = 1152 if MM_DT == "bf16" else 768      # max token chunk resident in SBUF
HC = 512                                       # down-proj output col chunk
HB = H // 128        # 16
IB = I // 128        # 11
TB = TA // 128       # 8


def _bc(ap, n):
    """Broadcast a trailing size-1 free dim of an AP to n (step 0)."""
    a = [list(x) for x in ap.ap]
    assert a[-1][1] == 1
    a[-1] = [0, n]
    return bass.AP(tensor=ap.tensor, offset=ap.offset, ap=a)


def _bcast_row(t, p, n, offset=0):
    """DRAM 1-D tensor AP broadcast across p partitions, n elements."""
    return bass.AP(tensor=t.tensor, offset=offset, ap=[[0, p], [1, n]])


F32R = mybir.dt.float32r
_MDT = {"bf16": BF16, "f32": F32, "f32r": F32R}


def _round_f32r(x):
    """Host-side replication of the on-chip f32->f32r rounding (RNE on the low
    8 mantissa bits, keeping 15 explicit mantissa bits)."""
    b = x.view(np.uint32).astype(np.uint64)
    lsb = (b >> np.uint64(8)) & np.uint64(1)
    r = (b + np.uint64(0x7F) + lsb) & ~np.uint64(0xFF)
    return r.astype(np.uint32).view(np.float32)


def build_phase_a(silu_unused=None):
    """RMSNorm (rstd only; rms_weight is folded into router/expert weights on
    the host), PE transpose, router matmul, top-2 combine weights."""
    out_dt = BF16 if MM_DT == "bf16" else F32
    nc = bacc.Bacc("TRN2", target_bir_lowering=False, debug=False)
    x = nc.dram_tensor("x", [TA, H], F32, kind="ExternalInput").ap()
    rt = nc.dram_tensor("rt", [H, E], F32, kind="ExternalInput").ap()   # (router*rms).T
    xnt = nc.dram_tensor("xnt", [H, TA], out_dt, kind="ExternalOutput").ap()
    cw = nc.dram_tensor("cw", [TA, E], F32, kind="ExternalOutput").ap()
    lg = nc.dram_tensor("lg", [TA, E], F32, kind="ExternalOutput").ap()

    x_r = x.rearrange("(tb p) h -> p tb h", p=128)
    xnt_r = xnt.rearrange("(hb p) t -> p hb t", p=128)

    with tile.TileContext(nc) as tc, ExitStack() as ctx:
        konst = ctx.enter_context(tc.tile_pool(name="konst", bufs=1))
        xp = ctx.enter_context(tc.tile_pool(name="xp", bufs=4))
        trp = ctx.enter_context(tc.tile_pool(name="trp", bufs=3))
        outp = ctx.enter_context(tc.tile_pool(name="outp", bufs=2))
        sm = ctx.enter_context(tc.tile_pool(name="sm", bufs=4))
        pers = ctx.enter_context(tc.tile_pool(name="pers", bufs=1))
        pst = ctx.enter_context(tc.tile_pool(name="pst", bufs=6, space="PSUM"))
        psl = ctx.enter_context(tc.tile_pool(name="psl", bufs=2, space="PSUM"))

        ident = konst.tile([128, 128], F32)
        make_identity(nc, ident)
        rwt = konst.tile([128, HB, E], F32)
        nc.sync.dma_start(out=rwt, in_=rt.rearrange("(hb p) e -> p hb e", p=128))
        epst = konst.tile([128, 1], F32)
        nc.vector.memset(epst, EPS)

        lg_all = pers.tile([128, TB, E], F32)

        for tt in range(TB):
            xt = xp.tile([128, H], F32, tag="xt")
            nc.sync.dma_start(out=xt, in_=x_r[:, tt, :])
            stats = sm.tile([128, 4, 6], F32, tag="stats")
            for sg in range(4):
                nc.vector.bn_stats(stats[:, sg, :], xt[:, sg * 512:(sg + 1) * 512])
            mv = sm.tile([128, 2], F32, tag="mv")
            nc.vector.bn_aggr(mv, stats)
            # mean(x^2) = var + mean^2 ; rstd = 1/sqrt(mean_sq + eps)
            msq = sm.tile([128, 1], F32, tag="msq")
            nc.vector.tensor_mul(msq, mv[:, 0:1], mv[:, 0:1])
            nc.vector.tensor_add(msq, msq, mv[:, 1:2])
            rstd = sm.tile([128, 1], F32, tag="rstd")
            nc.scalar.activation(rstd, msq, ACTF.Sqrt, bias=epst, scale=1.0)
            nc.vector.reciprocal(rstd, rstd)
            xn = xp.tile([128, H], F32, tag="xn")
            nc.vector.tensor_scalar_mul(xn, xt, rstd)

            tr = trp.tile([128, HB, 128], F32)
            for hb in range(HB):
                ptr = pst.tile([128, 128], F32)
                nc.tensor.transpose(ptr, xn[:, hb * 128:(hb + 1) * 128], ident)
                if hb % 2 == 0:
                    nc.scalar.copy(tr[:, hb, :], ptr)
                else:
                    nc.vector.tensor_copy(out=tr[:, hb, :], in_=ptr)

            if out_dt == F32:
                nc.sync.dma_start(out=xnt_r[:, :, tt * 128:(tt + 1) * 128], in_=tr)
            else:
                trc = outp.tile([128, HB, 128], out_dt)
                nc.vector.tensor_copy(trc, tr)
                nc.sync.dma_start(out=xnt_r[:, :, tt * 128:(tt + 1) * 128], in_=trc)

            lg_ps = psl.tile([128, E], F32)
            for hb in range(HB):
                nc.tensor.matmul(lg_ps, tr[:, hb, :], rwt[:, hb, :],
                                 start=(hb == 0), stop=(hb == HB - 1))
            nc.any.tensor_copy(out=lg_all[:, tt, :], in_=lg_ps)

        # batched softmax-top2 combine weights over [128, TB, E]
        m = sm.tile([128, TB, 1], F32, tag="m")
        nc.vector.tensor_reduce(out=m, in_=lg_all, op=ALU.max, axis=AX.X)
        d = pers.tile([128, TB, E], F32, tag="d")
        nc.vector.tensor_tensor(out=d, in0=lg_all, in1=_bc(m, E), op=ALU.subtract)
        p = pers.tile([128, TB, E], F32, tag="p")
        nc.scalar.activation(p, d, ACTF.Exp)
        p1 = sm.tile([128, TB, 1], F32, tag="p1")
        nc.vector.tensor_reduce(out=p1, in_=p, op=ALU.max, axis=AX.X)
        m1 = pers.tile([128, TB, E], F32, tag="m1")
        nc.vector.tensor_tensor(out=m1, in0=p, in1=_bc(p1, E), op=ALU.is_ge)
        pm = pers.tile([128, TB, E], F32, tag="pm")
        nc.vector.tensor_tensor(out=pm, in0=p, in1=m1, op=ALU.mult)
        nc.vector.tensor_tensor(out=pm, in0=p, in1=pm, op=ALU.subtract)
        p2 = sm.tile([128, TB, 1], F32, tag="p2")
        nc.vector.tensor_reduce(out=p2, in_=pm, op=ALU.max, axis=AX.X)
        m2 = pers.tile([128, TB, E], F32, tag="m2")
        nc.vector.tensor_tensor(out=m2, in0=pm, in1=_bc(p2, E), op=ALU.is_ge)
        den = sm.tile([128, TB, 1], F32, tag="den")
        nc.vector.tensor_tensor(out=den, in0=p1, in1=p2, op=ALU.add)
        inv = sm.tile([128, TB, 1], F32, tag="inv")
        nc.vector.reciprocal(inv, den)
        sel = pers.tile([128, TB, E], F32, tag="sel")
        nc.vector.tensor_tensor(out=sel, in0=m1, in1=m2, op=ALU.add)
        cwt = pers.tile([128, TB, E], F32, tag="cwt")
        nc.vector.tensor_tensor(out=cwt, in0=p, in1=sel, op=ALU.mult)
        nc.vector.tensor_tensor(out=cwt, in0=cwt, in1=_bc(inv, E), op=ALU.mult)

        nc.sync.dma_start(out=lg.rearrange("(tb p) e -> p tb e", p=128), in_=lg_all)
        nc.sync.dma_start(out=cw.rearrange("(tb p) e -> p tb e", p=128), in_=cwt)

    nc.compile()
    return nc


def _col_groups(n):
    """Split n columns into groups <=512, each >=256 when possible (f32r
    matmuls need output free dim >=256 for the fast path)."""
    sizes, left = [], n
    while left > 512:
        take = 512 if left - 512 >= 256 else left - 256
        sizes.append(take)
        left -= take
    sizes.append(left)
    groups, n0 = [], 0
    for s in sizes:
        groups.append((n0, s))
        n0 += s
    return groups


def _chunks(cap, chunk):
    out, off = [], 0
    while off < cap:
        n = min(chunk, cap - off)
        out.append((off, n))
        off += n
    return out


def build_phase_b():
    mdt = _MDT[MM_DT]
    nc = bacc.Bacc("TRN2", target_bir_lowering=False, debug=False)
    xet = nc.dram_tensor("xet", [H, CAP], mdt, kind="ExternalInput").ap()
    # wg+wu packed/interleaved per i-tile: [IB, HB, 128, 256] (cols 0:128 = gate)
    wgu = nc.dram_tensor("wgu", [IB, HB, 128, 256], mdt, kind="ExternalInput").ap()
    wd = nc.dram_tensor("wd", [I, H], mdt, kind="ExternalInput").ap()
    cwv = nc.dram_tensor("cwv", [CAP], F32, kind="ExternalInput").ap()
    y = nc.dram_tensor("y", [CAP, H], F32, kind="ExternalOutput").ap()

    xet_r = xet.rearrange("(hb p) t -> p hb t", p=128)
    wd_r = wd.rearrange("(ib p) h -> p ib h", p=128)
    y_r = y.rearrange("(tb p) h -> p tb h", p=128)

    NHC = H // HC

    with tile.TileContext(nc) as tc, ExitStack() as ctx:
        xp = ctx.enter_context(tc.tile_pool(name="xp", bufs=2 if mdt == BF16 else 1))
        hp = ctx.enter_context(tc.tile_pool(name="hp", bufs=1))
        wp = ctx.enter_context(tc.tile_pool(name="wp", bufs=2))
        dp = ctx.enter_context(tc.tile_pool(name="dp", bufs=2))
        sp = ctx.enter_context(tc.tile_pool(name="sp", bufs=2))
        cwp = ctx.enter_context(tc.tile_pool(name="cwp", bufs=2))
        yp = ctx.enter_context(tc.tile_pool(name="yp", bufs=4))
        pg = ctx.enter_context(tc.tile_pool(name="pg", bufs=1, space="PSUM"))
        py = ctx.enter_context(tc.tile_pool(name="py", bufs=2, space="PSUM"))

        for (c_off, n_c) in _chunks(CAP, CHUNK):
            cgs = _col_groups(n_c)
            TBC = n_c // 128
            xet_sb = xp.tile([128, HB, n_c], mdt, tag="xet")
            for h4 in range(0, HB, 4):
                nc.sync.dma_start(out=xet_sb[:, h4:h4 + 4, :],
                                  in_=xet_r[:, h4:h4 + 4, c_off:c_off + n_c])
            cwb = cwp.tile([128, n_c], F32, tag="cwb")
            nc.sync.dma_start(out=cwb,
                              in_=bass.AP(tensor=cwv.tensor, offset=c_off,
                                          ap=[[0, 128], [1, n_c]]))
            ht_sb = hp.tile([128, IB, n_c], mdt, tag="ht")
            for i in range(IB):
                wgut = wp.tile([128, HB, 256], mdt, tag="wgu")
                for h4 in range(0, HB, 4):
                    nc.sync.dma_start(
                        out=wgut[:, h4:h4 + 4, :],
                        in_=wgu[i, h4:h4 + 4].rearrange("hb p k -> p hb k"))
                gt = pg.tile([128, n_c], F32, tag="g", padded_shape=[128, CHUNK])
                ut = pg.tile([128, n_c], F32, tag="u", padded_shape=[128, CHUNK])
                for (n0, nn) in cgs:
                    for hb in range(HB):
                        nc.tensor.matmul(gt[:, n0:n0 + nn], wgut[:, hb, 0:128],
                                         xet_sb[:, hb, n0:n0 + nn],
                                         start=(hb == 0), stop=(hb == HB - 1))
                for (n0, nn) in cgs:
                    for hb in range(HB):
                        nc.tensor.matmul(ut[:, n0:n0 + nn], wgut[:, hb, 128:256],
                                         xet_sb[:, hb, n0:n0 + nn],
                                         start=(hb == 0), stop=(hb == HB - 1))
                sg = sp.tile([128, n_c], F32, tag="sg", padded_shape=[128, CHUNK])
                if USE_SILU_LUT:
                    nc.scalar.activation(sg, gt, ACTF.Silu)
                else:
                    nc.scalar.activation(sg, gt, ACTF.Sigmoid)
                    nc.vector.tensor_mul(sg, sg, gt)
                hu = sp.tile([128, n_c], F32, tag="hu", padded_shape=[128, CHUNK])
                nc.vector.tensor_mul(hu, sg, ut)
                nc.vector.tensor_mul(ht_sb[:, i, :], hu, cwb)

            for hc in range(NHC):
                wdt = dp.tile([128, IB, HC], mdt, tag="wd")
                nc.sync.dma_start(out=wdt, in_=wd_r[:, :, hc * HC:(hc + 1) * HC])
                for t in range(TBC):
                    if c_off + t * 128 >= MAX_USED:
                        continue
                    y_ps = py.tile([128, HC], F32, tag="y")
                    for i in range(IB):
                        nc.tensor.matmul(y_ps, ht_sb[:, i, t * 128:(t + 1) * 128],
                                         wdt[:, i, :],
                                         start=(i == 0), stop=(i == IB - 1))
                    ysb = yp.tile([128, HC], F32, tag="ysb")
                    nc.any.tensor_copy(out=ysb, in_=y_ps)
                    nc.sync.dma_start(
                        out=y_r[:, c_off // 128 + t, hc * HC:(hc + 1) * HC], in_=ysb)

    nc.compile()
    return nc


_programs = {}
_last_results = {}


def _trace_kwargs():
    if os.environ.get("MOE_TRACE", "0") != "1":
        return {}
    return {"trace": True}


def _get_program(name):
    if name not in _programs:
        _programs[name] = build_phase_a() if name == "a" else build_phase_b()
    return _programs[name]


_wcache = {}


def _prep_weights(w_gate, w_up, w_down, rms_weight):
    key = (id(w_gate), id(w_up), id(w_down), id(rms_weight))
    _wcache["rms"] = np.asarray(rms_weight, dtype=np.float32)
    if _wcache.get("key") == key:
        return _wcache["val"]
    mnp = ml_dtypes.bfloat16 if MM_DT == "bf16" else np.float32

    def blocks(wT):
        # [H, I] -> [IB, HB, 128, 128]
        return wT.reshape(HB, 128, IB, 128).transpose(2, 0, 1, 3)

    val = []
    rms = None
    for e in range(E):
        wgT = (np.asarray(w_gate)[e] * _wcache["rms"][None, :]).T.astype(mnp)
        wuT = (np.asarray(w_up)[e] * _wcache["rms"][None, :]).T.astype(mnp)
        wgu = np.ascontiguousarray(
            np.concatenate([blocks(wgT), blocks(wuT)], axis=-1))
        val.append({
            "wgu": wgu,
            "wd": np.ascontiguousarray(np.asarray(w_down)[e].T).astype(mnp),
        })
    _wcache["key"] = key
    _wcache["val"] = val
    return val


def kernel(hidden_states, rms_weight, router_weight, w_gate, w_up, w_down):
    _last_results.clear()
    x = np.ascontiguousarray(np.asarray(hidden_states), dtype=np.float32).reshape(T, H)
    rw = np.asarray(rms_weight, dtype=np.float32)
    rt = np.ascontiguousarray((np.asarray(router_weight, dtype=np.float32)
                               * rw[None, :]).T)

    nc_a = _get_program("a")
    core_ids = list(range(NCORES))
    in_maps_a = [{"x": x[c * TA:(c + 1) * TA], "rt": rt}
                 for c in core_ids]
    res_a = run_bass_kernel_spmd(nc_a, in_maps_a, core_ids, **_trace_kwargs())
    _last_results["a"] = res_a

    logits = np.concatenate([r["lg"] for r in res_a.results], axis=0)
    cw_full = np.concatenate([r["cw"] for r in res_a.results], axis=0)
    xnt_full = np.concatenate([r["xnt"] for r in res_a.results], axis=1)  # [H, T]

    idxs = [np.nonzero(cw_full[:, e] > 0)[0] for e in range(E)]
    wmaps = _prep_weights(w_gate, w_up, w_down, rms_weight)
    mnp = ml_dtypes.bfloat16 if MM_DT == "bf16" else np.float32

    out = np.zeros((T, H), dtype=np.float32)
    nc_b = _get_program("b")
    rounds = max(1, max((len(ix) + MAX_USED - 1) // MAX_USED for ix in idxs))
    for r in range(rounds):
        in_maps_b = []
        for e in range(E):
            idx = idxs[e][r * MAX_USED:(r + 1) * MAX_USED]
            xet = np.zeros((H, CAP), dtype=mnp)
            cwv = np.zeros((CAP,), dtype=np.float32)
            if len(idx):
                xet[:, :len(idx)] = xnt_full[:, idx]
                cwv[:len(idx)] = cw_full[idx, e]
            in_maps_b.append({"xet": xet, "cwv": cwv, **wmaps[e]})
        res_b = run_bass_kernel_spmd(nc_b, in_maps_b, core_ids, **_trace_kwargs())
        _last_results.setdefault("b", []).append(res_b)
        for e in range(E):
            idx = idxs[e][r * MAX_USED:(r + 1) * MAX_USED]
            if len(idx):
                out[idx] += res_b.results[e]["y"][:len(idx)]

    return out.reshape(B, S, H), logits.reshape(B, S, E)
